# revision 1
# baseline (speedup 1.0000x reference)
"""Bass/Trainium2 kernel for nn_DotsGenerator (scatter_memory).

Strategy (8 NeuronCores, SPMD):
  - 512 crops sharded 64/core along the crop axis (host slices zero-haloed
    42x42 crops out of the image = the shard step; per-core data differs,
    the program is identical).
  - Crop loads: batched contiguous-strip DMAs build the conv1 im2col
    directly (shift baked into the source offset; windowed rhs APs skip
    the junk columns), spread across the gpsimd and sync queues.
  - conv1: single K=27 f32r matmul per N-chunk, output M-DUPLICATED to
    partitions 64-114 with the duplicate evacuated one column left.
  - conv2: 6 paired-shift K=115 f32r matmuls per crop (two 3x3 taps per
    pass via the shifted duplicate; zero weight rows bridge 51-63), its
    own output M-duplicated so the two evacuations write even/odd pixels
    straight into the conv3 layout (no rearrange DMAs).
  - conv3 (40x40 full-map): 800 accumulating K=128 bf16 matmuls (FWL)
    over ft[64*parity + ch, crop*800 + pixpair], N = 64 crops.
  - Each core also DMA-copies its 135-row band of the image to its output.
  - Host assembles the bands and applies the 512*17*9 dot values.
"""

import sys

sys.path.insert(0, "/opt/trn_rl_repo")

import numpy as np
import ml_dtypes

import concourse.bass as bass
import concourse.bacc as bacc
import concourse.tile as tile
import concourse.mybir as mybir
from concourse.bass_utils import run_bass_kernel_spmd

F32 = mybir.dt.float32
F32R = mybir.dt.float32r
BF16 = mybir.dt.bfloat16

NCORES = 8
NGT = 512
PC = NGT // NCORES  # crops per core = 64
CROP = 40
PAD = 42  # padded map 42x42
PIX = CROP * CROP  # 1600
NPAIR = PC // 2  # crop pairs per core = 32
BAND_H = 1080 // NCORES  # 135 rows of output per core
IMG_H, IMG_W = 1080, 1920
EPS = 1e-5
NCH = 51
J3 = PIX // 2  # 800 pixel-pairs for conv3
HALO = PAD * PAD  # 1764 elems per haloed crop channel
STRIP = (CROP - 1) * PAD + CROP  # 1678 contiguous elems cover a shifted window
W3BLK = 50  # conv3 weight chunks per DMA block

DOT_LIST = np.array(
    [(30, 20), (20, 30), (10, 20), (20, 10), (40, 20), (34, 34), (20, 40),
     (6, 34), (0, 20), (6, 6), (20, 0), (34, 6), (17, 20), (23, 20),
     (20, 17), (20, 23), (20, 20)], dtype=np.int64)  # [17,2] (dy,dx)
DIRS = np.array([(dy, dx) for dy in (-1, 0, 1) for dx in (-1, 0, 1)],
                dtype=np.int64)  # [9,2]


def _emit(ctx, tc, io, n_pairs):
    """Emit the per-core program. io: dict of DRAM APs."""
    nc = tc.nc
    pc = 2 * n_pairs
    crops = io["crops"]        # [3, pc, 42, 42] f32r (zero halo)
    w1r = io["w1r"]            # [27, 115]  f32r (im2col lhsT, M-duplicated)
    w2r = io["w2r"]            # [6, 115, 115] f32r (paired-shift, M-dup)
    w3r = io["w3r"]            # [J3, 128, 51] bf16
    b12 = io["b12"]            # [128, 2] f32
    b3 = io["b3"]              # [128, 1] f32
    vals_out = io["vals_out"]  # [51, pc] f32 out
    band_src = io["band_src"]  # [3, BAND_H, 1920] f32
    out_band = io["out_band"]  # [3, BAND_H, 1920] f32 out

    # ---- pools ----
    consts = ctx.enter_context(tc.tile_pool(name="consts", bufs=1))
    cin_pool = ctx.enter_context(tc.tile_pool(name="cin", bufs=2))
    pad_pool = ctx.enter_context(tc.tile_pool(name="pad1", bufs=4))
    ft_pool = ctx.enter_context(tc.tile_pool(name="ft", bufs=1))
    w3_pool = ctx.enter_context(tc.tile_pool(name="w3", bufs=2))
    ps_pool = ctx.enter_context(tc.tile_pool(name="psum", bufs=int(__import__("os").environ.get("PSBUFS", "4")), space="PSUM"))
    ps3_pool = ctx.enter_context(tc.tile_pool(name="psum3", bufs=1,
                                              space="PSUM"))
    out_pool = ctx.enter_context(tc.tile_pool(name="outs", bufs=1))

    # ---- constants in SBUF ----
    w1t = consts.tile([128, 115], F32R)       # rows 0-26: im2col lhsT
    nc.sync.dma_start(w1t[0:27, :], w1r[:, :])
    w2t = consts.tile([128, 6 * 115], F32R)   # rows 0-114
    w2t_v = w2t.rearrange("p (s o) -> p s o", s=6)
    nc.sync.dma_start(w2t_v[0:115], w2r.rearrange("s i o -> i s o"))
    b12t = consts.tile([128, 2], F32)
    nc.sync.dma_start(b12t[:, :], b12[:, :])
    b3t = consts.tile([128, 1], F32)
    nc.sync.dma_start(b3t[:, :], b3[:, :])

    # ---- persistent conv3 feature store [128, pc*J3] bf16 ----
    ft = ft_pool.tile([128, pc * J3], BF16)
    # zero once: the channel-pad rows (51-63, 115-127) must stay clean, as
    # garbage there would poison the K=128 contraction (0-weight * NaN).
    nc.gpsimd.memset(ft[:, :], 0.0)

    shifts = [(ky, kx) for ky in range(3) for kx in range(3)]
    NCHUNK = 4          # 4 N-chunks of 10 output rows (400 px, >=256 for f32r)
    ROWS_PER = CROP // NCHUNK
    GRP = 4             # crops loaded per batched DMA group

    def emit_load(c):
        # ---- batched im2col crop loads (see header) ----------------------
        g = c // GRP
        cin = cin_pool.tile([128, GRP * HALO], F32R, tag="cin")
        cin_v = cin.rearrange("p (n j) -> p n j", n=GRP)
        csrc = crops.rearrange("c n h w -> c n (h w)")[
            :, g * GRP:(g + 1) * GRP, :]
        for s, (ky, kx) in enumerate(shifts):
            off = ky * PAD + kx
            eng = (nc.gpsimd, nc.sync, nc.scalar)[s % 3]
            eng.dma_start(
                cin_v[3 * s:3 * s + 3, :, 0:STRIP],
                csrc[:, :, off:off + STRIP])
        return cin_v

    def emit_conv1(c, cin_v):
        # ---- conv1: K=27, M-duplicated out (rows 0-50 and 64-114) --------
        # band2 is evacuated one column left, so conv2 can read shift
        # (ky,kx) on rows 0-50 and (ky,kx+1) on rows 64-114 with ONE AP.
        pad1 = pad_pool.tile([128, PAD * PAD], F32R)
        pad1_v = pad1.rearrange("p (h w) -> p h w", h=PAD)
        if "no_memset" not in DBG:
            nc.gpsimd.memset(pad1[0:115, :].bitcast(F32), 0.0)
        cin_c = cin_v[:, c % GRP].rearrange("p (h w) -> p h w", w=PAD)
        for ci in range(NCHUNK):
            r0 = ci * ROWS_PER
            ps = ps_pool.tile([128, ROWS_PER * CROP], F32)
            ps_v = ps.rearrange("p (h w) -> p h w", h=ROWS_PER)
            nc.tensor.matmul(
                ps[0:115, :], w1t[0:27, :],
                cin_c[0:27, r0:r0 + ROWS_PER, 0:CROP],
                start=True, stop=True)
            nc.scalar.activation(
                pad1_v[0:NCH, 1 + r0:1 + r0 + ROWS_PER, 1:41],
                ps_v[0:NCH],
                mybir.ActivationFunctionType.Relu, bias=b12t[0:NCH, 0:1])
            nc.vector.tensor_scalar(
                pad1_v[64:64 + NCH, 1 + r0:1 + r0 + ROWS_PER, 0:40],
                ps_v[64:64 + NCH],
                b12t[64:64 + NCH, 0:1], 0.0,
                mybir.AluOpType.add, mybir.AluOpType.max)
        return pad1_v

    def emit_conv2(c, pad1_v):
        # ---- conv2: 6 paired passes, K=115, M-duplicated out -------------
        # The two evacuations write even pixels (rows 0-50, stride-2 PSUM
        # read) and odd pixels (rows 64-114) DIRECTLY into the conv3
        # layout: no SBUF-to-SBUF rearrange DMAs at all.
        for ci in range(NCHUNK):
            r0 = ci * ROWS_PER
            npx = ROWS_PER * CROP
            ps = ps_pool.tile([128, npx], F32)
            ps_v = ps.rearrange("p (h w) -> p h w", h=ROWS_PER)
            for si, (ky, kx) in enumerate(
                    (ky, kx) for ky in range(3) for kx in (0, 2)):
                nc.tensor.matmul(
                    ps_v[0:115],
                    w2t[0:115, si * 115:(si + 1) * 115],
                    pad1_v[0:115, ky + r0:ky + r0 + ROWS_PER, kx:kx + CROP],
                    start=(si == 0), stop=(si == 5))
            ps_par = ps.rearrange("p (j r) -> p j r", r=2)
            j0 = c * J3 + ci * (npx // 2)
            nc.scalar.activation(
                ft[0:NCH, j0:j0 + npx // 2], ps_par[0:NCH, :, 0],
                mybir.ActivationFunctionType.Relu, bias=b12t[0:NCH, 1:2])
            nc.vector.tensor_scalar(
                ft[64:64 + NCH, j0:j0 + npx // 2], ps_par[64:64 + NCH, :, 1],
                b12t[64:64 + NCH, 1:2], 0.0,
                mybir.AluOpType.add, mybir.AluOpType.max)

    # Software-pipelined emission: conv1(c) is emitted BEFORE conv2(c-1)
    # so the PE always has conv2 work queued while conv1's evacuations
    # land in pad1 -- the PE never waits on a fresh evacuation.
    skip12 = ("no_conv1" in DBG) or ("no_conv2" in DBG)
    prev = None
    cin_v = None
    for c in range(pc + 1):
        if c < pc:
            if c % GRP == 0:
                cin_v = emit_load(c)
            if not skip12:
                pad1_cur = emit_conv1(c, cin_v)
        if c >= 1 and not skip12:
            emit_conv2(c - 1, prev)
        if c < pc and not skip12:
            prev = pad1_cur

    # ---- output band passthrough copy (pure DMA; emitted after the crop
    # loop so the sync queue serves the latency-critical strip loads first) --
    if "no_band" not in DBG:
        nc.gpsimd.dma_start(out_band[:, :, :], band_src[:, :, :])

    # ---- conv3: J3 accumulating K=128 matmuls, N = pc crops ----
    skip3 = "no_conv3" in DBG
    ps3 = ps3_pool.tile([128, pc], F32)
    if skip3:
        nc.gpsimd.memset(ps3[:, :], 0.0)
    ft_v = ft.rearrange("p (n j) -> p n j", j=J3)
    n_blk = 0 if skip3 else J3 // W3BLK
    for bi in range(n_blk):
        w3t = w3_pool.tile([128, W3BLK * NCH], BF16)
        nc.sync.dma_start(
            w3t.rearrange("p (n c) -> p n c", n=W3BLK),
            w3r[bi * W3BLK:(bi + 1) * W3BLK].rearrange("n p c -> p n c"))
        for k in range(W3BLK):
            j = bi * W3BLK + k
            nc.tensor.matmul(ps3[0:NCH, :],
                             w3t[:, k * NCH:(k + 1) * NCH],
                             ft_v[:, :, j],
                             start=(j == 0), stop=(j == J3 - 1))

    # relu(x + b3) then clip to 255
    ov = out_pool.tile([128, pc], F32)
    nc.scalar.activation(ov[0:NCH, :], ps3[0:NCH, :],
                         mybir.ActivationFunctionType.Relu, bias=b3t[0:NCH, :])
    nc.vector.tensor_scalar_min(ov[0:NCH, :], ov[0:NCH, :], 255.0)
    nc.sync.dma_start(vals_out[:, :], ov[0:NCH, :])


_CACHE = {}
DBG = set()          # ablation flags for cost-model analysis
RUN_KWARGS = {}     # test harness may set {"trace": True} for profiling
LAST_RESULTS = None


def _build(n_pairs=NPAIR):
    if n_pairs in _CACHE:
        return _CACHE[n_pairs]
    pc = 2 * n_pairs
    nc = bacc.Bacc("TRN2", target_bir_lowering=False, debug=False,
                   num_devices=NCORES)
    io = {
        "crops": nc.dram_tensor("crops", [3, pc, PAD, PAD], F32R,
                                kind="ExternalInput").ap(),
        "w1r": nc.dram_tensor("w1r", [27, 115], F32R,
                              kind="ExternalInput").ap(),
        "w2r": nc.dram_tensor("w2r", [6, 115, 115], F32R,
                              kind="ExternalInput").ap(),
        "w3r": nc.dram_tensor("w3r", [J3, 128, NCH], BF16,
                              kind="ExternalInput").ap(),
        "b12": nc.dram_tensor("b12", [128, 2], F32,
                              kind="ExternalInput").ap(),
        "b3": nc.dram_tensor("b3", [128, 1], F32,
                             kind="ExternalInput").ap(),
        "band_src": nc.dram_tensor("band_src", [3, BAND_H, IMG_W], F32,
                                   kind="ExternalInput").ap(),
        "vals_out": nc.dram_tensor("vals_out", [NCH, pc], F32,
                                   kind="ExternalOutput").ap(),
        "out_band": nc.dram_tensor("out_band", [3, BAND_H, IMG_W], F32,
                                   kind="ExternalOutput").ap(),
    }
    from contextlib import ExitStack
    with tile.TileContext(nc) as tc, ExitStack() as ctx:
        _emit(ctx, tc, io, n_pairs)
    nc.compile()
    _CACHE[n_pairs] = nc
    return nc


def _fold(w, g, b, m, v):
    scale = g / np.sqrt(v + EPS)
    return w * scale[:, None, None, None], (b - m * scale).astype(np.float32)


def _prep_weights(w1, g1, b1, m1, v1, w2, g2, b2, m2, v2, w3, g3, b3, m3, v3):
    w1f, b1f = _fold(w1, g1, b1, m1, v1)  # [51,3,3,3]
    w2f, b2f = _fold(w2, g2, b2, m2, v2)  # [51,51,3,3]
    w3f, b3f = _fold(w3, g3, b3, m3, v3)  # [51,51,40,40]
    # per-shift lhsT [K=in, M=out]
    w1c = np.ascontiguousarray(
        w1f.transpose(2, 3, 1, 0).reshape(27, NCH)).astype(np.float32)
    w1r = np.zeros((27, 115), np.float32)
    w1r[:, 0:NCH] = w1c
    w1r[:, 64:64 + NCH] = w1c
    w2c = np.ascontiguousarray(
        w2f.transpose(2, 3, 1, 0).reshape(3, 3, NCH, NCH)).astype(np.float32)
    w2r = np.zeros((6, 115, 115), np.float32)
    for ky in range(3):
        for kxg, kx in enumerate((0, 2)):
            p = ky * 2 + kxg
            w2r[p, 0:NCH, 0:NCH] = w2c[ky, kx]
            w2r[p, 0:NCH, 64:64 + NCH] = w2c[ky, kx]
            if kx + 1 < 3:
                w2r[p, 64:64 + NCH, 0:NCH] = w2c[ky, kx + 1]
                w2r[p, 64:64 + NCH, 64:64 + NCH] = w2c[ky, kx + 1]
    # conv3: rows (par, c) for pixel pair j -> pixel p = 2j+par
    w3p = w3f.transpose(2, 3, 1, 0).reshape(J3, 2, NCH, NCH)  # [j,par,c,o]
    w3r = np.zeros((J3, 2, 64, NCH), np.float32)
    w3r[:, :, :NCH, :] = w3p
    w3r = w3r.reshape(J3, 128, NCH).astype(ml_dtypes.bfloat16)
    b12 = np.zeros((128, 2), np.float32)
    b12[0:NCH, 0] = b1f
    b12[64:64 + NCH, 0] = b1f
    b12[0:NCH, 1] = b2f
    b12[64:64 + NCH, 1] = b2f
    b3v = np.zeros((128, 1), np.float32)
    b3v[0:NCH, 0] = b3f
    return w1r, w2r, w3r, b12, b3v


def kernel(image, targets, w1, g1, b1, m1, v1, w2, g2, b2, m2, v2,
           w3, g3, b3, m3, v3):
    image = np.asarray(image, np.float32)
    targets = np.asarray(targets)
    w1r, w2r, w3r, b12, b3v = _prep_weights(
        np.asarray(w1, np.float32), np.asarray(g1, np.float32),
        np.asarray(b1, np.float32), np.asarray(m1, np.float32),
        np.asarray(v1, np.float32),
        np.asarray(w2, np.float32), np.asarray(g2, np.float32),
        np.asarray(b2, np.float32), np.asarray(m2, np.float32),
        np.asarray(v2, np.float32),
        np.asarray(w3, np.float32), np.asarray(g3, np.float32),
        np.asarray(b3, np.float32), np.asarray(m3, np.float32),
        np.asarray(v3, np.float32))

    lt = targets[:, :2].astype(np.int64)  # [512,2] (y,x)
    # shard: crops (host gather = crop-axis shard) + image bands
    in_maps = []
    for c in range(NCORES):
        ci = lt[c * PC:(c + 1) * PC]
        crops = np.zeros((3, PC, PAD, PAD), np.float32)
        for k, (y, x) in enumerate(ci):
            crops[:, k, 1:41, 1:41] = image[:, y:y + CROP, x:x + CROP]
        in_maps.append({
            "crops": crops,
            "w1r": w1r, "w2r": w2r, "w3r": w3r, "b12": b12, "b3": b3v,
            "band_src": np.ascontiguousarray(
                image[:, c * BAND_H:(c + 1) * BAND_H, :]),
        })

    nc = _build()
    res_obj = run_bass_kernel_spmd(nc, in_maps, list(range(NCORES)),
                                   **RUN_KWARGS)
    globals()["LAST_RESULTS"] = res_obj
    res = res_obj.results

    out = np.empty_like(image)
    vals = np.empty((NGT, NCH), np.float32)
    for c in range(NCORES):
        out[:, c * BAND_H:(c + 1) * BAND_H, :] = res[c]["out_band"]
        vals[c * PC:(c + 1) * PC] = res[c]["vals_out"].T
    # host scatter of the dot values (unshard/assembly step)
    v = vals.reshape(NGT, 17, 3)
    coords = (lt[:, None, None, :] + DOT_LIST[None, :, None, :]
              + DIRS[None, None, :, :]).reshape(-1, 2)  # [512*17*9, 2]
    vflat = np.broadcast_to(v[:, :, None, :],
                            (NGT, 17, 9, 3)).reshape(-1, 3)
    out[:, coords[:, 0], coords[:, 1]] = vflat.T
    return out



# revision 4
# speedup vs baseline: 1.9245x; 1.9245x over previous
"""Bass/Trainium2 kernel for nn_DotsGenerator (scatter_memory).

Strategy (8 NeuronCores, SPMD), v2:
  - 512 crops sharded 64/core along the crop axis (host slices zero-haloed
    42x42 bf16 crops out of the image; per-core data differs, program
    identical).
  - Crop loads: batched contiguous-strip DMAs build the conv1 im2col
    directly (shift baked into the source offset), split across the sync
    and scalar HWDGE queues.
  - conv1: single K=27 bf16 matmul per N-chunk, M=51, one relu+bias evac
    per chunk (Act/DVE alternating) into the padded map pad1 rows 0-50.
  - pad1's +1-column-shifted duplicate (rows 64-114) is made by ONE flat
    SBUF->SBUF DMA: dst[64+c, j] = src[c, j+1]; the row-41 wraparound
    cell picks up the next row's zero border, which is exactly the
    padding value it needs.
  - conv2: pixel-PAIR M-packing. Output column = pixel pair; M rows 0-50
    hold the even pixel's channels, rows 64-114 the odd pixel's. 6
    accumulating K=115 passes (3 ky x 2 column offsets) using the dual
    shifted copies cover all 9 taps for both parities: 4800 PE
    column-cycles per crop (2x fewer than pixel-per-column). One evac
    per 400-pair chunk writes ft rows 0-128 straight into the conv3
    layout; zero M-columns make the pad rows 51-63/115-127 true zeros,
    so ft needs no memset at all.
  - conv3 (40x40 full-map): 800 accumulating K=128 bf16 matmuls over
    ft[64*parity + ch, crop*800 + pair], N = 64 crops. w3 lives in DRAM
    as [128, 800*51] so every block DMA moves >=5KB-contiguous rows;
    blocks stream through the crop loop (prefetch) and the conv3 phase.
  - Each core also DMA-copies its 135-row band of the image to its
    output (f32 passthrough, exact).
  - Host assembles the bands and applies the 512*17*9 dot values.
"""

import sys

sys.path.insert(0, "/opt/trn_rl_repo")

import numpy as np
import ml_dtypes

import concourse.bass as bass
import concourse.bacc as bacc
import concourse.tile as tile
import concourse.mybir as mybir
from concourse.bass_utils import run_bass_kernel_spmd

F32 = mybir.dt.float32
BF16 = mybir.dt.bfloat16

NCORES = 8
NGT = 512
PC = NGT // NCORES  # crops per core = 64
CROP = 40
PAD = 42  # padded map 42x42
PIX = CROP * CROP  # 1600
NPAIR = PC // 2
BAND_H = 1080 // NCORES  # 135 rows of output per core
IMG_H, IMG_W = 1080, 1920
EPS = 1e-5
NCH = 51
J3 = PIX // 2  # 800 pixel-pairs for conv3
HALO = PAD * PAD  # 1764 elems per haloed crop channel
STRIP = (CROP - 1) * PAD + CROP  # 1678 contiguous elems cover a window
GRP = 8  # crops loaded per batched DMA group
W3BLK = 50  # conv3 pixel-pairs per weight block
NBLK = J3 // W3BLK  # 16 blocks
W3BUFS = 6

DOT_LIST = np.array(
    [(30, 20), (20, 30), (10, 20), (20, 10), (40, 20), (34, 34), (20, 40),
     (6, 34), (0, 20), (6, 6), (20, 0), (34, 6), (17, 20), (23, 20),
     (20, 17), (20, 23), (20, 20)], dtype=np.int64)  # [17,2] (dy,dx)
DIRS = np.array([(dy, dx) for dy in (-1, 0, 1) for dx in (-1, 0, 1)],
                dtype=np.int64)  # [9,2]


def _emit(ctx, tc, io, n_pairs):
    """Emit the per-core program. io: dict of DRAM APs."""
    nc = tc.nc
    pc = 2 * n_pairs
    crops = io["crops"]        # [3, pc, 42, 42] bf16 (zero halo)
    w1r = io["w1r"]            # [27, 51]  bf16 (im2col lhsT)
    w2r = io["w2r"]            # [6, 115, 128] bf16 (pixel-pair lhsT)
    w3r = io["w3r"]            # [128, J3*51] bf16 (partition-major)
    b12 = io["b12"]            # [128, 2] f32
    b3 = io["b3"]              # [128, 1] f32
    vals_out = io["vals_out"]  # [51, pc] f32 out
    band_src = io["band_src"]  # [3, BAND_H, 1920] f32
    out_band = io["out_band"]  # [3, BAND_H, 1920] f32 out

    # ---- pools ----
    consts = ctx.enter_context(tc.tile_pool(name="consts", bufs=1))
    cin_pool = ctx.enter_context(tc.tile_pool(name="cin", bufs=2))
    pad_pool = ctx.enter_context(tc.tile_pool(name="pad1", bufs=4))
    ft_pool = ctx.enter_context(tc.tile_pool(name="ft", bufs=1))
    w3_pool = ctx.enter_context(tc.tile_pool(name="w3", bufs=W3BUFS))
    ps1_pool = ctx.enter_context(tc.tile_pool(name="psum1", bufs=4,
                                              space="PSUM"))
    ps2_pool = ctx.enter_context(tc.tile_pool(name="psum2", bufs=2,
                                              space="PSUM"))
    ps3_pool = ctx.enter_context(tc.tile_pool(name="psum3", bufs=1,
                                              space="PSUM"))
    out_pool = ctx.enter_context(tc.tile_pool(name="outs", bufs=1))

    # ---- constants in SBUF ----
    w1t = consts.tile([128, NCH], BF16)       # rows 0-26: im2col lhsT
    nc.sync.dma_start(w1t[0:27, :], w1r[:, :])
    w2t = consts.tile([128, 6 * 128], BF16)   # rows 0-114
    w2t_v = w2t.rearrange("p (s o) -> p s o", s=6)
    nc.sync.dma_start(w2t_v[0:115], w2r.rearrange("s i o -> i s o"))
    b12t = consts.tile([128, 2], F32)
    nc.sync.dma_start(b12t[:, :], b12[:, :])
    b3t = consts.tile([128, 1], F32)
    nc.sync.dma_start(b3t[:, :], b3[:, :])

    # ---- persistent conv3 feature store [128, pc*J3] bf16 (no memset:
    # every row is written by the conv2 evacs, pad rows as true zeros) ----
    ft = ft_pool.tile([128, pc * J3], BF16)
    ft_v = ft.rearrange("p (n j) -> p n j", j=J3)

    shifts = [(ky, kx) for ky in range(3) for kx in range(3)]
    NCHUNK = 4          # conv1: 4 N-chunks of 10 output rows (400 px)
    ROWS_PER = CROP // NCHUNK

    w3_tiles = []

    def emit_w3_load(bi):
        w3t = w3_pool.tile([128, W3BLK * NCH], BF16, tag="w3")
        nc.sync.dma_start(
            w3t[:, :], w3r[:, bi * W3BLK * NCH:(bi + 1) * W3BLK * NCH])
        w3_tiles.append(w3t)

    def emit_load(c):
        # ---- batched im2col crop loads (see header) ----------------------
        g = c // GRP
        cin = cin_pool.tile([128, GRP * HALO], BF16, tag="cin")
        cin_v = cin.rearrange("p (n j) -> p n j", n=GRP)
        csrc = crops.rearrange("c n h w -> c n (h w)")[
            :, g * GRP:(g + 1) * GRP, :]
        for s, (ky, kx) in enumerate(shifts):
            off = ky * PAD + kx
            eng = (nc.sync, nc.scalar)[s % 2]
            eng.dma_start(
                cin_v[3 * s:3 * s + 3, :, 0:STRIP],
                csrc[:, :, off:off + STRIP])
        return cin_v

    def emit_conv1(c, cin_v):
        # ---- conv1: K=27, M=51, one evac per chunk -----------------------
        pad1 = pad_pool.tile([128, PAD * PAD], BF16)
        pad1_v = pad1.rearrange("p (h w) -> p h w", h=PAD)
        if c < 4 and "no_memset" not in DBG:
            # one-time border zeroing per pool buffer; the interior is
            # fully rewritten every crop, the halo cells only ever hold 0.
            nc.gpsimd.memset(pad1[0:115, :], 0.0)
        cin_c = cin_v[:, c % GRP].rearrange("p (h w) -> p h w", w=PAD)
        for ci in range(NCHUNK):
            r0 = ci * ROWS_PER
            ps = ps1_pool.tile([128, ROWS_PER * CROP], F32)
            ps_v = ps.rearrange("p (h w) -> p h w", h=ROWS_PER)
            nc.tensor.matmul(
                ps[0:NCH, :], w1t[0:27, :],
                cin_c[0:27, r0:r0 + ROWS_PER, 0:CROP],
                start=True, stop=True)
            dst = pad1_v[0:NCH, 1 + r0:1 + r0 + ROWS_PER, 1:41]
            if ci % 2 == 0:
                nc.scalar.activation(
                    dst, ps_v[0:NCH],
                    mybir.ActivationFunctionType.Relu, bias=b12t[0:NCH, 0:1])
            else:
                nc.vector.tensor_scalar(
                    dst, ps_v[0:NCH], b12t[0:NCH, 0:1], 0.0,
                    mybir.AluOpType.add, mybir.AluOpType.max)
        # +1-col shifted duplicate via flat SBUF->SBUF DMA (see header)
        nc.gpsimd.dma_start(pad1[64:64 + NCH, 0:HALO - 1],
                            pad1[0:NCH, 1:HALO])
        return pad1.rearrange("p (h w2 t) -> p h w2 t", h=PAD, t=2)

    def emit_conv2(c, pad1_p):
        # ---- conv2: pixel-pair M-packed, 6 accumulating K=115 passes ----
        for ci in range(2):
            r0 = 20 * ci
            ps = ps2_pool.tile([128, 400], F32)
            for si in range(6):
                ky, b2 = si // 2, si % 2
                nc.tensor.matmul(
                    ps[0:128, :],
                    w2t_v[0:115, si],
                    pad1_p[0:115, r0 + ky:r0 + ky + 20, b2:b2 + 20, 0:1],
                    start=(si == 0), stop=(si == 5))
            j0 = c * J3 + ci * 400
            if ci == 0:
                nc.scalar.activation(
                    ft[0:128, j0:j0 + 400], ps[0:128, :],
                    mybir.ActivationFunctionType.Relu, bias=b12t[:, 1:2])
            else:
                nc.vector.tensor_scalar(
                    ft[0:128, j0:j0 + 400], ps[0:128, :],
                    b12t[:, 1:2], 0.0,
                    mybir.AluOpType.add, mybir.AluOpType.max)

    # Software-pipelined emission: conv1(c) is emitted BEFORE conv2(c-1)
    # so the PE always has conv2 work queued while conv1's evacuations
    # land in pad1.
    skip12 = ("no_conv1" in DBG) or ("no_conv2" in DBG)
    skip3 = "no_conv3" in DBG
    prev = None
    cin_v = None
    for c in range(pc + 1):
        if c < pc:
            if c % GRP == 0:
                cin_v = emit_load(c)
            if c % 4 == 1 and c // 4 < W3BUFS and not skip3:
                # prefetch the first W3BUFS w3 blocks (no buffer-reuse waits,
                # so they can't stall the sync queue mid-loop)
                emit_w3_load(c // 4)
            if c in (2, 6, 10) and "no_band" not in DBG:
                ch = (c - 2) // 4
                nc.gpsimd.dma_start(out_band[ch], band_src[ch])
            if not skip12:
                pad1_cur = emit_conv1(c, cin_v)
        if c >= 1 and not skip12:
            emit_conv2(c - 1, prev)
        if c < pc and not skip12:
            prev = pad1_cur

    # ---- conv3: J3 accumulating K=128 matmuls, N = pc crops ----
    ps3 = ps3_pool.tile([128, pc], F32)
    if skip3:
        nc.gpsimd.memset(ps3[:, :], 0.0)
    n_blk = 0 if skip3 else NBLK
    for bi in range(W3BUFS, n_blk):
        # remaining w3 blocks stream in as their buffers free up under the
        # conv3 matmuls (sync queue has nothing else left to do)
        emit_w3_load(bi)
    for bi in range(n_blk):
        w3t = w3_tiles[bi]
        for k in range(W3BLK):
            j = bi * W3BLK + k
            nc.tensor.matmul(ps3[0:NCH, :],
                             w3t[:, k * NCH:(k + 1) * NCH],
                             ft_v[:, :, j],
                             start=(j == 0), stop=(j == J3 - 1))

    # relu(x + b3) then clip to 255
    ov = out_pool.tile([128, pc], F32)
    nc.scalar.activation(ov[0:NCH, :], ps3[0:NCH, :],
                         mybir.ActivationFunctionType.Relu, bias=b3t[0:NCH, :])
    nc.vector.tensor_scalar_min(ov[0:NCH, :], ov[0:NCH, :], 255.0)
    nc.sync.dma_start(vals_out[:, :], ov[0:NCH, :])


_CACHE = {}
DBG = set()          # ablation flags for cost-model analysis
RUN_KWARGS = {}     # test harness may set {"trace": True} for profiling
LAST_RESULTS = None


def _build(n_pairs=NPAIR):
    if n_pairs in _CACHE:
        return _CACHE[n_pairs]
    pc = 2 * n_pairs
    nc = bacc.Bacc("TRN2", target_bir_lowering=False, debug=False,
                   num_devices=NCORES)
    io = {
        "crops": nc.dram_tensor("crops", [3, pc, PAD, PAD], BF16,
                                kind="ExternalInput").ap(),
        "w1r": nc.dram_tensor("w1r", [27, NCH], BF16,
                              kind="ExternalInput").ap(),
        "w2r": nc.dram_tensor("w2r", [6, 115, 128], BF16,
                              kind="ExternalInput").ap(),
        "w3r": nc.dram_tensor("w3r", [128, J3 * NCH], BF16,
                              kind="ExternalInput").ap(),
        "b12": nc.dram_tensor("b12", [128, 2], F32,
                              kind="ExternalInput").ap(),
        "b3": nc.dram_tensor("b3", [128, 1], F32,
                             kind="ExternalInput").ap(),
        "band_src": nc.dram_tensor("band_src", [3, BAND_H, IMG_W], F32,
                                   kind="ExternalInput").ap(),
        "vals_out": nc.dram_tensor("vals_out", [NCH, pc], F32,
                                   kind="ExternalOutput").ap(),
        "out_band": nc.dram_tensor("out_band", [3, BAND_H, IMG_W], F32,
                                   kind="ExternalOutput").ap(),
    }
    from contextlib import ExitStack
    with tile.TileContext(nc) as tc, ExitStack() as ctx:
        _emit(ctx, tc, io, n_pairs)
    nc.compile()
    _CACHE[n_pairs] = nc
    return nc


def _fold(w, g, b, m, v):
    scale = g / np.sqrt(v + EPS)
    return w * scale[:, None, None, None], (b - m * scale).astype(np.float32)


def _prep_weights(w1, g1, b1, m1, v1, w2, g2, b2, m2, v2, w3, g3, b3, m3, v3):
    w1f, b1f = _fold(w1, g1, b1, m1, v1)  # [51,3,3,3]
    w2f, b2f = _fold(w2, g2, b2, m2, v2)  # [51,51,3,3]
    w3f, b3f = _fold(w3, g3, b3, m3, v3)  # [51,51,40,40]
    w1r = np.ascontiguousarray(
        w1f.transpose(2, 3, 1, 0).reshape(27, NCH)).astype(ml_dtypes.bfloat16)
    # conv2 pixel-pair lhsT: pass si = 2*ky + b; M cols 0-50 even px,
    # 64-114 odd px; K rows 0-50 copy1 (padded col c), 64-114 copy2 (c+1).
    w2c = np.ascontiguousarray(
        w2f.transpose(2, 3, 1, 0))  # [ky, kx, in, out]
    w2r = np.zeros((6, 115, 128), np.float32)
    for ky in range(3):
        a, b_ = 2 * ky, 2 * ky + 1
        w2r[a, 0:NCH, 0:NCH] = w2c[ky, 0]
        w2r[a, 64:64 + NCH, 0:NCH] = w2c[ky, 1]
        w2r[a, 64:64 + NCH, 64:64 + NCH] = w2c[ky, 0]
        w2r[b_, 0:NCH, 0:NCH] = w2c[ky, 2]
        w2r[b_, 0:NCH, 64:64 + NCH] = w2c[ky, 1]
        w2r[b_, 64:64 + NCH, 64:64 + NCH] = w2c[ky, 2]
    w2r = w2r.astype(ml_dtypes.bfloat16)
    # conv3: row (64*parity + c_in), col (pair j * 51 + out)
    w3p = w3f.transpose(2, 3, 1, 0).reshape(J3, 2, NCH, NCH)  # [j,par,ci,o]
    w3r = np.zeros((2, 64, J3, NCH), np.float32)
    w3r[:, :NCH] = w3p.transpose(1, 2, 0, 3)
    w3r = np.ascontiguousarray(
        w3r.reshape(128, J3 * NCH)).astype(ml_dtypes.bfloat16)
    b12 = np.zeros((128, 2), np.float32)
    b12[0:NCH, 0] = b1f
    b12[0:NCH, 1] = b2f
    b12[64:64 + NCH, 1] = b2f
    b3v = np.zeros((128, 1), np.float32)
    b3v[0:NCH, 0] = b3f
    return w1r, w2r, w3r, b12, b3v


def kernel(image, targets, w1, g1, b1, m1, v1, w2, g2, b2, m2, v2,
           w3, g3, b3, m3, v3):
    image = np.asarray(image, np.float32)
    targets = np.asarray(targets)
    w1r, w2r, w3r, b12, b3v = _prep_weights(
        np.asarray(w1, np.float32), np.asarray(g1, np.float32),
        np.asarray(b1, np.float32), np.asarray(m1, np.float32),
        np.asarray(v1, np.float32),
        np.asarray(w2, np.float32), np.asarray(g2, np.float32),
        np.asarray(b2, np.float32), np.asarray(m2, np.float32),
        np.asarray(v2, np.float32),
        np.asarray(w3, np.float32), np.asarray(g3, np.float32),
        np.asarray(b3, np.float32), np.asarray(m3, np.float32),
        np.asarray(v3, np.float32))

    image_bf = image.astype(ml_dtypes.bfloat16)
    lt = targets[:, :2].astype(np.int64)  # [512,2] (y,x)
    # shard: crops (host gather = crop-axis shard) + image bands
    in_maps = []
    for c in range(NCORES):
        ci = lt[c * PC:(c + 1) * PC]
        crops = np.zeros((3, PC, PAD, PAD), ml_dtypes.bfloat16)
        for k, (y, x) in enumerate(ci):
            crops[:, k, 1:41, 1:41] = image_bf[:, y:y + CROP, x:x + CROP]
        in_maps.append({
            "crops": crops,
            "w1r": w1r, "w2r": w2r, "w3r": w3r, "b12": b12, "b3": b3v,
            "band_src": np.ascontiguousarray(
                image[:, c * BAND_H:(c + 1) * BAND_H, :]),
        })

    nc = _build()
    res_obj = run_bass_kernel_spmd(nc, in_maps, list(range(NCORES)),
                                   **RUN_KWARGS)
    globals()["LAST_RESULTS"] = res_obj
    res = res_obj.results

    out = np.empty_like(image)
    vals = np.empty((NGT, NCH), np.float32)
    for c in range(NCORES):
        out[:, c * BAND_H:(c + 1) * BAND_H, :] = res[c]["out_band"]
        vals[c * PC:(c + 1) * PC] = res[c]["vals_out"].T
    # host scatter of the dot values (unshard/assembly step)
    v = vals.reshape(NGT, 17, 3)
    coords = (lt[:, None, None, :] + DOT_LIST[None, :, None, :]
              + DIRS[None, None, :, :]).reshape(-1, 2)  # [512*17*9, 2]
    vflat = np.broadcast_to(v[:, :, None, :],
                            (NGT, 17, 9, 3)).reshape(-1, 3)
    out[:, coords[:, 0], coords[:, 1]] = vflat.T
    return out


# revision 22
# speedup vs baseline: 2.4809x; 1.2891x over previous
"""Bass/Trainium2 kernel for nn_DotsGenerator (scatter_memory).

Strategy (8 NeuronCores, SPMD), v2:
  - 512 crops sharded 64/core along the crop axis (host slices zero-haloed
    42x42 bf16 crops out of the image; per-core data differs, program
    identical).
  - Crop loads: batched contiguous-strip DMAs build the conv1 im2col
    directly (shift baked into the source offset), split across the sync
    and scalar HWDGE queues.
  - conv1: single K=27 bf16 matmul per N-chunk, M=51, one relu+bias evac
    per chunk (Act/DVE alternating) into the padded map pad1 rows 0-50.
  - pad1's +1-column-shifted duplicate (rows 64-114) is made by ONE flat
    SBUF->SBUF DMA: dst[64+c, j] = src[c, j+1]; the row-41 wraparound
    cell picks up the next row's zero border, which is exactly the
    padding value it needs.
  - conv2: pixel-PAIR M-packing. Output column = pixel pair; M rows 0-50
    hold the even pixel's channels, rows 64-114 the odd pixel's. 6
    accumulating K=115 passes (3 ky x 2 column offsets) using the dual
    shifted copies cover all 9 taps for both parities: 4800 PE
    column-cycles per crop (2x fewer than pixel-per-column). One evac
    per 400-pair chunk writes ft rows 0-128 straight into the conv3
    layout; zero M-columns make the pad rows 51-63/115-127 true zeros,
    so ft needs no memset at all.
  - conv3 (40x40 full-map): 800 accumulating K=128 bf16 matmuls over
    ft[64*parity + ch, crop*800 + pair], N = 64 crops. w3 lives in DRAM
    as [128, 800*51] so every block DMA moves >=5KB-contiguous rows;
    blocks stream through the crop loop (prefetch) and the conv3 phase.
  - Each core also DMA-copies its 135-row band of the image to its
    output (f32 passthrough, exact).
  - Host assembles the bands and applies the 512*17*9 dot values.
"""

import sys

sys.path.insert(0, "/opt/trn_rl_repo")

import numpy as np
import ml_dtypes

import concourse.bass as bass
import concourse.bacc as bacc
import concourse.tile as tile
import concourse.mybir as mybir
from concourse.bass_utils import run_bass_kernel_spmd

F32 = mybir.dt.float32
BF16 = mybir.dt.bfloat16

NCORES = 8
NGT = 512
PC = NGT // NCORES  # crops per core = 64
CROP = 40
PAD = 42  # padded map 42x42
PIX = CROP * CROP  # 1600
NPAIR = PC // 2
BAND_H = 1080 // NCORES  # 135 rows of output per core
IMG_H, IMG_W = 1080, 1920
EPS = 1e-5
NCH = 51
J3 = PIX // 2  # 800 pixel-pairs for conv3
STRIP = (CROP - 1) * PAD + CROP  # 1678 contiguous elems cover a window
SPAN = 1680  # per-crop strip span in cin (42*40, factorable for the AP view)
GRP = 8  # crops loaded per batched DMA group
W3BLK = 50  # conv3 pixel-pairs per weight block
NBLK = J3 // W3BLK  # 16 blocks
W3BUFS = 7

DOT_LIST = np.array(
    [(30, 20), (20, 30), (10, 20), (20, 10), (40, 20), (34, 34), (20, 40),
     (6, 34), (0, 20), (6, 6), (20, 0), (34, 6), (17, 20), (23, 20),
     (20, 17), (20, 23), (20, 20)], dtype=np.int64)  # [17,2] (dy,dx)
DIRS = np.array([(dy, dx) for dy in (-1, 0, 1) for dx in (-1, 0, 1)],
                dtype=np.int64)  # [9,2]


def _emit(ctx, tc, io, n_pairs):
    """Emit the per-core program. io: dict of DRAM APs."""
    nc = tc.nc
    pc = 2 * n_pairs
    strips = io["strips"]      # [36, pc, 1678] bf16 im2col strip rows
    w1r = io["w1r"]            # [36, 128] bf16 (pixel-pair lhsT)
    w2r = io["w2r"]            # [6, 115, 128] bf16 (pixel-pair lhsT)
    w3r = io["w3r"]            # [128, J3*51] bf16 (partition-major)
    b12 = io["b12"]            # [128, 2] f32
    b3 = io["b3"]              # [128, 1] f32
    vals_out = io["vals_out"]  # [51, pc] f32 out
    band_src = io["band_src"]  # [3, BAND_H, 1920] f32
    out_band = io["out_band"]  # [3, BAND_H, 1920] f32 out

    # ---- pools ----
    consts = ctx.enter_context(tc.tile_pool(name="consts", bufs=1))
    cin_pool = ctx.enter_context(tc.tile_pool(name="cin", bufs=2))
    pad_pool = ctx.enter_context(tc.tile_pool(name="pad1", bufs=4))
    ft_pool = ctx.enter_context(tc.tile_pool(name="ft", bufs=1))
    w3_pool = ctx.enter_context(tc.tile_pool(name="w3", bufs=W3BUFS))
    ps1_pool = ctx.enter_context(tc.tile_pool(name="psum1", bufs=4,
                                              space="PSUM"))
    ps2_pool = ctx.enter_context(tc.tile_pool(name="psum2", bufs=2,
                                              space="PSUM"))
    ps3_pool = ctx.enter_context(tc.tile_pool(name="psum3", bufs=1,
                                              space="PSUM"))
    out_pool = ctx.enter_context(tc.tile_pool(name="outs", bufs=1))

    # ---- constants in SBUF (DMAs scheduled by first use: w1t ahead of the
    # group-0 strips on sync, b12 ahead of the scalar strips, w2t on the
    # gpsimd queue behind group-0's strips, b3 whenever) ----
    w1t = consts.tile([128, 128], BF16)       # rows 0-35: pixel-pair lhsT
    nc.sync.dma_start(w1t[0:36, :], w1r[:, :])
    w2t = consts.tile([128, 6 * 128], BF16)   # rows 0-114
    w2t_v = w2t.rearrange("p (s o) -> p s o", s=6)
    b12t = consts.tile([128, 2], F32)
    nc.scalar.dma_start(b12t[:, :], b12[:, :])
    b3t = consts.tile([128, 1], F32)

    # ---- persistent conv3 feature store [128, pc*J3] bf16 (no memset:
    # every row is written by the conv2 evacs, pad rows as true zeros) ----
    ft = ft_pool.tile([128, pc * J3], BF16)
    ft_v = ft.rearrange("p (n j) -> p n j", j=J3)

    w3_tiles = []

    def emit_w3_load(bi, eng=None):
        w3t = w3_pool.tile([128, W3BLK * NCH], BF16, tag="w3")
        (eng or nc.sync).dma_start(
            w3t[:, :], w3r[:, bi * W3BLK * NCH:(bi + 1) * W3BLK * NCH])
        w3_tiles.append(w3t)

    cin_views = {}

    def emit_load(g):
        # ---- batched im2col crop loads: the host pre-lays the 36 strip
        # rows (kx, ch, ky) contiguously, so HALF A GROUP is one 3D DMA ----
        cin = cin_pool.tile([128, GRP * SPAN], BF16, tag="cin")
        cin_v = cin.rearrange("p (n j) -> p n j", n=GRP)
        for h, eng in enumerate((nc.sync, nc.scalar)):
            n0 = g * GRP + 4 * h
            eng.dma_start(
                cin_v[0:36, 4 * h:4 * h + 4, 0:STRIP],
                strips[:, n0:n0 + 4, :])
        cin_views[g] = cin.rearrange("p (n h w2 t) -> p n h w2 t",
                                     n=GRP, h=CROP, t=2)

    def emit_conv1(c, cin_p):
        # ---- conv1: pixel-pair M-packed, K=36, one matmul per 20-row
        # chunk (N=400 pairs); even px in PSUM rows 0-50, odd in 64-114 ---
        pad1 = pad_pool.tile([128, PAD * PAD], BF16)
        pad1_q = pad1.rearrange("p (h w2 t) -> p h w2 t", h=PAD, t=2)
        if c < 4 and "no_memset" not in DBG:
            # one-time border zeroing per pool buffer; the interior is
            # fully rewritten every crop, the halo cells only ever hold 0.
            nc.gpsimd.memset(pad1[0:115, :], 0.0)
        for ci in range(2):
            r0 = 20 * ci
            ps = ps1_pool.tile([128, 400], F32)
            ps_v = ps.rearrange("p (h w) -> p h w", h=20)
            nc.tensor.matmul(
                ps[0:128, :], w1t[0:36, :],
                cin_p[0:36, c % GRP, r0:r0 + 20, 0:20, 0:1],
                start=True, stop=True)
            # even px (rr, 2i) -> padded (rr+1, 2i+1); odd -> (rr+1, 2i+2)
            dst_e = pad1_q[0:NCH, 1 + r0:21 + r0, 0:20, 1:2]
            dst_o = pad1_q[0:NCH, 1 + r0:21 + r0, 1:21, 0:1]
            ea, eb = (0, 1) if ci == 0 else (1, 0)
            for pick, dst, src in ((ea, dst_e, ps_v[0:NCH]),
                                   (eb, dst_o, ps_v[64:64 + NCH])):
                if pick == 0:
                    nc.scalar.activation(
                        dst, src,
                        mybir.ActivationFunctionType.Relu,
                        bias=b12t[0:NCH, 0:1])
                else:
                    nc.vector.tensor_scalar(
                        dst, src, b12t[64:64 + NCH, 0:1], 0.0,
                        mybir.AluOpType.add, mybir.AluOpType.max)
        # +1-col shifted duplicate via flat SBUF->SBUF DMA (see header).
        # On sync/HWDGE: keeping it off the Pool engine keeps the serial
        # SWDGE pipeline (strips/band/memsets) out of the conv2 dep chain.
        nc.sync.dma_start(pad1[64:64 + NCH, 0:PAD * PAD - 1],
                          pad1[0:NCH, 1:PAD * PAD])
        return pad1_q

    def emit_conv2(c, pad1_p):
        # ---- conv2: pixel-pair M-packed, 6 accumulating K=115 passes ----
        for ci in range(2):
            r0 = 20 * ci
            ps = ps2_pool.tile([128, 400], F32)
            for si in range(6):
                ky, b2 = si // 2, si % 2
                nc.tensor.matmul(
                    ps[0:128, :],
                    w2t_v[0:115, si],
                    pad1_p[0:115, r0 + ky:r0 + ky + 20, b2:b2 + 20, 0:1],
                    start=(si == 0), stop=(si == 5))
            j0 = c * J3 + ci * 400
            if ci == 0:
                nc.scalar.activation(
                    ft[0:128, j0:j0 + 400], ps[0:128, :],
                    mybir.ActivationFunctionType.Relu, bias=b12t[:, 1:2])
            else:
                nc.vector.tensor_scalar(
                    ft[0:128, j0:j0 + 400], ps[0:128, :],
                    b12t[:, 1:2], 0.0,
                    mybir.AluOpType.add, mybir.AluOpType.max)

    # Software-pipelined emission at depth 2: conv1(c) and conv1(c+1) are
    # both queued before conv2(c), so the evac -> copy2-DMA chain for a
    # crop hides under ~5us of already-queued PE work.
    skip12 = ("no_conv1" in DBG) or ("no_conv2" in DBG)
    skip3 = "no_conv3" in DBG
    DEPTH = 2
    pads = {}
    for c in range(pc + DEPTH):
        if c < pc:
            if c == 0:
                emit_load(0)
            if c % GRP == 5 and c // GRP + 1 < pc // GRP:
                # prefetch next group's strips; at c%8==5 the target cin
                # buffer's WAR (group g-1's conv1 reads) is already resolved,
                # so the strips fire immediately instead of blocking a queue
                emit_load(c // GRP + 1)
            if c == 1:
                nc.gpsimd.dma_start(
                    w2t_v[0:115], w2r.rearrange("s i o -> i s o"))
            if c == 3:
                nc.sync.dma_start(b3t[:, :], b3[:, :])
            if c % 4 == 1 and c // 4 < W3BUFS and not skip3:
                # prefetch the first W3BUFS w3 blocks (no buffer-reuse waits,
                # so they can't stall the sync queue mid-loop)
                emit_w3_load(c // 4)
            if c in (2, 6, 10, 14, 18, 22) and "no_band" not in DBG:
                k = (c - 2) // 4
                ch, lo, hi = k // 2, (k % 2) * 68, (68, BAND_H)[k % 2]
                nc.gpsimd.dma_start(out_band[ch, lo:hi], band_src[ch, lo:hi])
            if not skip12:
                pads[c] = emit_conv1(c, cin_views[c // GRP])
        if c >= DEPTH and not skip12:
            emit_conv2(c - DEPTH, pads.pop(c - DEPTH))

    # ---- conv3: J3 accumulating K=128 matmuls, N = pc crops ----
    ps3 = ps3_pool.tile([128, pc], F32)
    if skip3:
        nc.gpsimd.memset(ps3[:, :], 0.0)
    n_blk = 0 if skip3 else NBLK
    for bi in range(W3BUFS, n_blk):
        # remaining w3 blocks stream in as their buffers free up under the
        # conv3 matmuls; alternate queues so supply outpaces consumption
        emit_w3_load(bi, (nc.sync, nc.scalar)[bi % 2])
    for bi in range(n_blk):
        w3t = w3_tiles[bi]
        for k in range(W3BLK):
            j = bi * W3BLK + k
            nc.tensor.matmul(ps3[0:NCH, :],
                             w3t[:, k * NCH:(k + 1) * NCH],
                             ft_v[:, :, j],
                             start=(j == 0), stop=(j == J3 - 1))

    # relu(x + b3) then clip to 255
    ov = out_pool.tile([128, pc], F32)
    nc.scalar.activation(ov[0:NCH, :], ps3[0:NCH, :],
                         mybir.ActivationFunctionType.Relu, bias=b3t[0:NCH, :])
    nc.vector.tensor_scalar_min(ov[0:NCH, :], ov[0:NCH, :], 255.0)
    nc.sync.dma_start(vals_out[:, :], ov[0:NCH, :])


_CACHE = {}
DBG = set()          # ablation flags for cost-model analysis
RUN_KWARGS = {}     # test harness may set {"trace": True} for profiling
LAST_RESULTS = None


def _build(n_pairs=NPAIR):
    if n_pairs in _CACHE:
        return _CACHE[n_pairs]
    pc = 2 * n_pairs
    nc = bacc.Bacc("TRN2", target_bir_lowering=False, debug=False,
                   num_devices=NCORES)
    io = {
        "strips": nc.dram_tensor("strips", [36, pc, STRIP], BF16,
                                 kind="ExternalInput").ap(),
        "w1r": nc.dram_tensor("w1r", [36, 128], BF16,
                              kind="ExternalInput").ap(),
        "w2r": nc.dram_tensor("w2r", [6, 115, 128], BF16,
                              kind="ExternalInput").ap(),
        "w3r": nc.dram_tensor("w3r", [128, J3 * NCH], BF16,
                              kind="ExternalInput").ap(),
        "b12": nc.dram_tensor("b12", [128, 2], F32,
                              kind="ExternalInput").ap(),
        "b3": nc.dram_tensor("b3", [128, 1], F32,
                             kind="ExternalInput").ap(),
        "band_src": nc.dram_tensor("band_src", [3, BAND_H, IMG_W], F32,
                                   kind="ExternalInput").ap(),
        "vals_out": nc.dram_tensor("vals_out", [NCH, pc], F32,
                                   kind="ExternalOutput").ap(),
        "out_band": nc.dram_tensor("out_band", [3, BAND_H, IMG_W], F32,
                                   kind="ExternalOutput").ap(),
    }
    from contextlib import ExitStack
    with tile.TileContext(nc) as tc, ExitStack() as ctx:
        _emit(ctx, tc, io, n_pairs)
    nc.compile()
    _CACHE[n_pairs] = nc
    return nc


def _fold(w, g, b, m, v):
    scale = g / np.sqrt(v + EPS)
    return w * scale[:, None, None, None], (b - m * scale).astype(np.float32)


def _prep_weights(w1, g1, b1, m1, v1, w2, g2, b2, m2, v2, w3, g3, b3, m3, v3):
    w1f, b1f = _fold(w1, g1, b1, m1, v1)  # [51,3,3,3]
    w2f, b2f = _fold(w2, g2, b2, m2, v2)  # [51,51,3,3]
    w3f, b3f = _fold(w3, g3, b3, m3, v3)  # [51,51,40,40]
    # conv1 pixel-pair lhsT [36, 128]: strip row r = 9*kx_s + 3*ch + ky;
    # M cols 0-50 even px (tap kx = kx_s), 64-114 odd px (tap kx = kx_s-1)
    w1r = np.zeros((36, 128), np.float32)
    for kxs in range(4):
        for ch in range(3):
            for ky in range(3):
                r = 9 * kxs + 3 * ch + ky
                if kxs <= 2:
                    w1r[r, 0:NCH] = w1f[:, ch, ky, kxs]
                if kxs >= 1:
                    w1r[r, 64:64 + NCH] = w1f[:, ch, ky, kxs - 1]
    w1r = w1r.astype(ml_dtypes.bfloat16)
    # conv2 pixel-pair lhsT: pass si = 2*ky + b; M cols 0-50 even px,
    # 64-114 odd px; K rows 0-50 copy1 (padded col c), 64-114 copy2 (c+1).
    w2c = np.ascontiguousarray(
        w2f.transpose(2, 3, 1, 0))  # [ky, kx, in, out]
    w2r = np.zeros((6, 115, 128), np.float32)
    for ky in range(3):
        a, b_ = 2 * ky, 2 * ky + 1
        w2r[a, 0:NCH, 0:NCH] = w2c[ky, 0]
        w2r[a, 64:64 + NCH, 0:NCH] = w2c[ky, 1]
        w2r[a, 64:64 + NCH, 64:64 + NCH] = w2c[ky, 0]
        w2r[b_, 0:NCH, 0:NCH] = w2c[ky, 2]
        w2r[b_, 0:NCH, 64:64 + NCH] = w2c[ky, 1]
        w2r[b_, 64:64 + NCH, 64:64 + NCH] = w2c[ky, 2]
    w2r = w2r.astype(ml_dtypes.bfloat16)
    # conv3: row (64*parity + c_in), col (pair j * 51 + out)
    w3p = w3f.transpose(2, 3, 1, 0).reshape(J3, 2, NCH, NCH)  # [j,par,ci,o]
    w3r = np.zeros((2, 64, J3, NCH), np.float32)
    w3r[:, :NCH] = w3p.transpose(1, 2, 0, 3)
    w3r = np.ascontiguousarray(
        w3r.reshape(128, J3 * NCH)).astype(ml_dtypes.bfloat16)
    b12 = np.zeros((128, 2), np.float32)
    b12[0:NCH, 0] = b1f
    b12[64:64 + NCH, 0] = b1f
    b12[0:NCH, 1] = b2f
    b12[64:64 + NCH, 1] = b2f
    b3v = np.zeros((128, 1), np.float32)
    b3v[0:NCH, 0] = b3f
    return w1r, w2r, w3r, b12, b3v


def kernel(image, targets, w1, g1, b1, m1, v1, w2, g2, b2, m2, v2,
           w3, g3, b3, m3, v3):
    image = np.asarray(image, np.float32)
    targets = np.asarray(targets)
    w1r, w2r, w3r, b12, b3v = _prep_weights(
        np.asarray(w1, np.float32), np.asarray(g1, np.float32),
        np.asarray(b1, np.float32), np.asarray(m1, np.float32),
        np.asarray(v1, np.float32),
        np.asarray(w2, np.float32), np.asarray(g2, np.float32),
        np.asarray(b2, np.float32), np.asarray(m2, np.float32),
        np.asarray(v2, np.float32),
        np.asarray(w3, np.float32), np.asarray(g3, np.float32),
        np.asarray(b3, np.float32), np.asarray(m3, np.float32),
        np.asarray(v3, np.float32))

    image_bf = image.astype(ml_dtypes.bfloat16)
    lt = targets[:, :2].astype(np.int64)  # [512,2] (y,x)
    # shard: im2col strips (host gather = crop-axis shard) + image bands.
    # strips[9*kx + 3*ch + ky, n, j] = halo43[ch, n, ky*42 + kx + j] where
    # halo43 is the 43x42 zero-padded crop (flat); one DMA covers 9 rows.
    in_maps = []
    for c in range(NCORES):
        ci = lt[c * PC:(c + 1) * PC]
        halo = np.zeros((3, PC, 43, PAD), ml_dtypes.bfloat16)
        for k, (y, x) in enumerate(ci):
            halo[:, k, 1:41, 1:41] = image_bf[:, y:y + CROP, x:x + CROP]
        flat = halo.reshape(3, PC, 43 * PAD)
        strips = np.empty((36, PC, STRIP), ml_dtypes.bfloat16)
        for kx in range(4):
            for ch in range(3):
                for ky in range(3):
                    off = ky * PAD + kx
                    strips[9 * kx + 3 * ch + ky] = \
                        flat[ch, :, off:off + STRIP]
        in_maps.append({
            "strips": strips,
            "w1r": w1r, "w2r": w2r, "w3r": w3r, "b12": b12, "b3": b3v,
            "band_src": np.ascontiguousarray(
                image[:, c * BAND_H:(c + 1) * BAND_H, :]),
        })

    nc = _build()
    res_obj = run_bass_kernel_spmd(nc, in_maps, list(range(NCORES)),
                                   **RUN_KWARGS)
    globals()["LAST_RESULTS"] = res_obj
    res = res_obj.results

    out = np.empty_like(image)
    vals = np.empty((NGT, NCH), np.float32)
    for c in range(NCORES):
        out[:, c * BAND_H:(c + 1) * BAND_H, :] = res[c]["out_band"]
        vals[c * PC:(c + 1) * PC] = res[c]["vals_out"].T
    # host scatter of the dot values (unshard/assembly step)
    v = vals.reshape(NGT, 17, 3)
    coords = (lt[:, None, None, :] + DOT_LIST[None, :, None, :]
              + DIRS[None, None, :, :]).reshape(-1, 2)  # [512*17*9, 2]
    vflat = np.broadcast_to(v[:, :, None, :],
                            (NGT, 17, 9, 3)).reshape(-1, 3)
    out[:, coords[:, 0], coords[:, 1]] = vflat.T
    return out
